# revision 69
# baseline (speedup 1.0000x reference)
"""ChannelAttention (XCA-style cross-covariance attention) TRN2 kernel.

Shapes (hardcoded): x [8, 128, 128, 128] f32 (B, H, W, C), C=128, heads=4,
hd=32, N = H*W = 16384 tokens per sample. 8 NeuronCores, data-parallel over
batch: core i processes sample i, weights replicated, no collectives.

Algebraic reduction: attention is over channels with l2-normalization over
the full token axis, so per sample everything collapses to
  S   = X^T [X|1] Gram stats:  S = X^T X (128x128), s = X^T 1 (128)
  G   = Wq^T S Wk + qb (x) (s^T Wk + N kb) + (Wq^T s) (x) kb
  sqq = diag(Wq^T S Wq) + 2 qb*(s^T Wq) + N qb^2   (same for k with kb)
  logits_h = exp(scale_h) * rsqrt(sqq) * G * rsqrt(sqk) ; A = softmax rows
  P   = blockdiag(A)^T @ proj_w ;  Wf = Wv P ;  bf = P^T v_bias + proj_b
  Y   = X @ Wf + bf
I/O is bf16 (host casts): x arrives as [16384, 130] bf16 with a ones column
(so one PE pass accumulates both S and s) padded to 130 for 4B-aligned rows;
host pre-permutes token rows so the on-chip PE transpose lands token-linear,
and Y is returned transposed [C, 16384] bf16 (host undoes it). All qkv bias
terms fold into PE accumulations via host-precomputed Wq*diag(2qb), N*qb^2
etc., and exp(-2*scale) is folded into the q-side operands so rq/rk come
from one sqrt + one reciprocal. Pass 2 computes Y^T = Wf^T X^T with Wf
stationary; the proj bias is a per-partition scalar fused into the
PSUM->SBUF copy. rsqrt drops the max(sq, EPS) guard: sq = sum of squares
over 16384 tokens is O(10^3) >> EPS for these inputs. The softmax row-sum
reciprocal is folded into proj_w rows.

Scheduling: engines execute in FIFO program order, so emission order is the
schedule. x arrives as a chain of DMA rings (~4 in flight, each ring's
trigger gated on ring j-4 landing) so SDMA bandwidth concentrates on the
data the PE needs next instead of round-robining the whole tensor. A
back-to-back dummy-matmul burst at t=0 + the gap-free gram stream trip the
HAM clock gate (which requires ~3.4us of GAP-FREE PE busy) during the DMA
dead window. PE transposes emit bf16 PSUM (is_transpose) so evacuations
read half the bytes — cheap enough that ALL pass-1 evacuations run on
Vector: Act issues no table-invalidating Copy during pass 1, so the Sqrt
table loaded once at startup is still live when the middle's sqrts need
it (saves ~1.3us of table reload on the critical path). The
last chunk's transposes, keepalive matmuls (HAM MID re-gates below ~70%
duty), and deferred evacuations are woven into the serial middle; softmax
runs without a reduce_max (exp bias = -exp(scale) bounds the exponent by
Cauchy-Schwarz; seed-0 worst row-max exponent ~-49 >> underflow). Pass 2
interleaves dummy matmuls from pair 5 on to hold PE duty while DMA-bound,
and writes block-major [8, C, 2048] so each block lands in a dense DRAM
span (host re-concatenates), as 16 half-block 256KB DMAs so the write
stream starts right after each matmul pair and the tail drains early. The
softmax mask lives inside G's PSUM accumulation (exact +-2^100 power-of-2
rank-1 matmuls, accumulated first so in-block entries cancel to zero), and
the logits stt reads the rk broadcast straight from PSUM.
"""

import os
import sys
import types

import numpy as np
import ml_dtypes

from concourse import bacc, mybir
import concourse.tile as tile
from concourse.bass_utils import run_bass_kernel_spmd
from concourse.masks import make_identity

F32 = mybir.dt.float32
BF16 = mybir.dt.bfloat16

B, H, W, C = 8, 128, 128, 128
NTOK = H * W          # 16384 tokens per sample
XCOL = C + 2          # x columns: C data + ones + pad
NT = NTOK // 128      # 128 token-tiles of 128 tokens
CHUNK = 16            # token-tiles per compute chunk
NCH = NT // CHUNK     # 8 compute chunks
DCN = 32              # token-tiles per DMA superchunk (8320B/descriptor)
NDC = NT // DCN       # 4 DMA superchunks
GRP = 4               # token-tiles per PSUM transpose group (1 bank)
NGRP = CHUNK // GRP   # 4 groups per chunk
GW = GRP * 128        # 512 tokens per transpose group
HEADS, HD = 4, 32
P2N = 512             # pass-2 tokens per matmul
NP2 = NTOK // P2N     # 32 pass-2 matmuls

LAST_EXEC_TIME_NS = None
_CACHED_NC = None


def _install_ntff_hook():
    """Register the axon NTFF profile hook if the image's antenv lacks it."""
    try:
        import antenv.axon_hooks  # noqa: F401
        return
    except ImportError:
        pass
    try:
        from trn_agent_boot.trn_boot import _ntff_profile_via_ctypes
        hook = _ntff_profile_via_ctypes("/opt/axon/libaxon_pjrt.so")
        mod = types.ModuleType("antenv.axon_hooks")
        mod.get_axon_ntff_profile_hook = lambda: hook
        sys.modules["antenv.axon_hooks"] = mod
    except Exception:
        pass


def build():
    nc = bacc.Bacc(None, target_bir_lowering=False, enable_partition_id=False)

    x_d = nc.declare_dram_parameter("x", [NTOK, XCOL], BF16, isOutput=False)
    # wpack columns: [0:256]=[Wq|Wk] [256:384]=Wk*diag(2kb) [384:512]=Wv
    #                [512:640]=Wq*diag(2qb)*diag(e2)
    #                [640:896]=[Wq*diag(e2) | Wk]  (one 256-wide prod mul)
    # with e2 = exp(-2*scale) per channel.
    wpack_d = nc.declare_dram_parameter("wpack", [C, 7 * C], BF16,
                                        isOutput=False)
    # rowpack: [0:128]=qb [128:256]=kb [256:384]=unused [384:512]=N*kb^2
    rowpack_d = nc.declare_dram_parameter("rowpack", [1, 4 * C], BF16,
                                          isOutput=False)
    nkb_d = nc.declare_dram_parameter("nkb_row", [1, C], F32, isOutput=False)
    pb_d = nc.declare_dram_parameter("pb_col", [C, 1], F32, isOutput=False)
    vb_d = nc.declare_dram_parameter("vb_col", [C, 1], BF16, isOutput=False)
    # colpack: col0 = N*qb^2*e2 (sqq sqrt bias), col1 = -exp(scale) (exp
    # bias: |l| <= exp(scale) by Cauchy-Schwarz, so exp(l-b) never overflows
    # and the seed-0 worst row-max exponent is ~-49, far above underflow),
    # col2 = N*kb^2 (sqk sqrt bias).
    colpack_d = nc.declare_dram_parameter("colpack", [C, 3], F32,
                                          isOutput=False)
    pw_d = nc.declare_dram_parameter("proj_w", [C, C], BF16, isOutput=False)
    # out is block-major [8, C, 2048]: each 2048-token block lands in one
    # dense 512KB DRAM span (vs 4KB chunks strided across 4MB), which the
    # HBM write path sustains measurably better. Host re-concatenates.
    out_d = nc.declare_dram_parameter("out", [NTOK // 2048, C, 2048], BF16,
                                      isOutput=True)

    # token row r = sc*4096 + p*32 + n -> partition p reads 32 contiguous
    # rows (32*260B = 8320B) per superchunk DMA. The host pre-permutes rows
    # so the PE-transposed column order comes out token-linear.
    x_t = x_d.ap().rearrange("(sc p n) c -> sc p n c", p=128, n=DCN)

    with tile.TileContext(nc) as tc:
        from contextlib import ExitStack
        with (
            tc.tile_pool(name="singles", bufs=1) as singles,
            tc.tile_pool(name="mid", bufs=1) as mid,
        ):
            ctx = ExitStack()
            mid_ctx = ExitStack()
            psum_s = ctx.enter_context(
                tc.tile_pool(name="psum_s", bufs=1, space="PSUM"))
            psum_mid = mid_ctx.enter_context(
                tc.tile_pool(name="psum_mid", bufs=3, space="PSUM"))

            # ---- x input as a chain of DMA rings -------------------------
            # SDMA round-robins ALL queued rings at packet granularity, so
            # blindly prefetching everything spreads bandwidth across the
            # whole tensor and the data the PE needs NEXT arrives late
            # (observed: PE starve at superchunk boundaries -> HAM
            # re-throttle). Instead keep ~3 rings in flight: ring j's
            # trigger waits (via a tiny SBUF->SBUF dma on the same queue)
            # until ring j-3 has landed, so bandwidth concentrates on data
            # in consumption order. The first pieces are small so grams
            # start early.
            ring_specs = [(0, 0, 2), (0, 2, 8), (0, 8, 16), (0, 16, 32)]
            for sc in range(1, NDC):
                ring_specs += [(sc, 0, 16), (sc, 16, 32)]
            dep_scratch = singles.tile([1, 2 * len(ring_specs)], BF16)
            rings = []
            for rj, (sc, a, b) in enumerate(ring_specs):
                if rj >= 6:
                    prev = rings[rj - 4][1]
                    nc.sync.dma_start(dep_scratch[:, 2 * rj:2 * rj + 2],
                                      prev[0:1, 0, 0:2])
                xh = singles.tile([128, b - a, XCOL], BF16, tag=f"xr{rj}")
                nc.sync.dma_start(xh[:], x_t[sc, :, a:b, :])
                rings.append(((sc, a, b), xh))

            def xtile(g):
                # global token-tile g -> (sbuf tile, row index within it)
                sc, nn = g // DCN, g % DCN
                for (rsc, a, b), xh in rings:
                    if rsc == sc and a <= nn < b:
                        return xh, nn - a
                raise AssertionError(g)

            # ---- weights on the Act HWDGE queue (Sync stays x-only) ------
            wpack = singles.tile([C, 7 * C], BF16)
            nc.scalar.dma_start(wpack[:], wpack_d[:, :])
            rowpack = singles.tile([1, 4 * C], BF16)
            nc.scalar.dma_start(rowpack[:], rowpack_d[:, :])
            nkb_row = singles.tile([1, C], F32)
            nc.scalar.dma_start(nkb_row[:], nkb_d[:, :])
            pb_col = singles.tile([C, 1], F32)
            nc.scalar.dma_start(pb_col[:], pb_d[:, :])
            vb_col = singles.tile([C, 1], BF16)
            nc.scalar.dma_start(vb_col[:], vb_d[:, :])
            colpack = singles.tile([C, 3], F32)
            nc.scalar.dma_start(colpack[:], colpack_d[:, :])
            pw_sb = singles.tile([C, C], BF16)
            nc.scalar.dma_start(pw_sb[:], pw_d[:, :])

            # ---- constants + PE warmup -----------------------------------
            ident_bf = singles.tile([128, 128], BF16)
            make_identity(nc, ident_bf[:])
            ones_col_bf = singles.tile([C, 1], BF16)
            nc.vector.memset(ones_col_bf[:], 1.0)
            ones_row_bf = singles.tile([1, C], BF16)
            nc.vector.memset(ones_row_bf[:], 1.0)
            one_one_bf = singles.tile([1, 1], BF16)
            nc.vector.memset(one_one_bf[:], 1.0)
            act_warm = singles.tile([1, 1], F32)
            nc.vector.memset(act_warm[:], 1.0)
            # load the Sqrt table during the dead startup window: pass-1's
            # Copy evacs don't touch the table, so the middle's sqrts run
            # without a 1.3us table load on the critical path.
            nc.scalar.sqrt(act_warm[:], act_warm[:])
            # block-diagonal mask folded into G's PSUM accumulation as two
            # exact power-of-2 rank-k matmuls: -2^100 everywhere (rank 1)
            # + 2^100 on the diagonal blocks (rank 4). Accumulated FIRST so
            # in-block entries cancel to exactly 0 before G lands; off-block
            # entries absorb G into -2^100 (they're masked anyway).
            mrow_neg = singles.tile([1, C], BF16)
            nc.vector.memset(mrow_neg[:], -(2.0 ** 50))
            mrow_pos = singles.tile([1, C], BF16)
            nc.vector.memset(mrow_pos[:], 2.0 ** 50)
            hrows = singles.tile([1, HEADS * C], BF16)
            nc.vector.memset(hrows[:], 0.0)
            for h in range(HEADS):
                nc.vector.memset(
                    hrows[:, h * C + h * HD:h * C + (h + 1) * HD], 2.0 ** 50)

            # s_ps doubles as the PE warmup / HAM-keepalive target: warmup
            # runs before the first gram resets it, keepalives run after the
            # middle has copied S out. The HAM SHORT window only trips after
            # ~3.4us of GAP-FREE PE busy (any gap resets it), so the burst
            # is one contiguous ~3.9us cold-rate stream; grams then start
            # warm at ~2x rate, which beats starting them cold immediately.
            s_ps = psum_s.tile([C, C + 1], F32)
            for _ in range(14):
                nc.tensor.matmul(s_ps[:, 0:C], lhsT=ident_bf[:],
                                 rhs=ident_bf[:], start=True, stop=True)

            def keepalive(lhs=None, n=3):
                for _ in range(n):
                    if lhs is None:
                        nc.tensor.matmul(s_ps[:, 0:C], lhsT=ident_bf[:],
                                         rhs=ident_bf[:], start=True,
                                         stop=True)
                    else:
                        nc.tensor.matmul(s_ps[0:1, 0:C], lhsT=lhs,
                                         rhs=ident_bf[:], start=True,
                                         stop=True)

            # Wv^T (x-independent) via PE transpose, during pass 1.
            wvT_ps = psum_mid.tile([C, C], F32, tag="mps")
            nc.tensor.matmul(wvT_ps[:], lhsT=wpack[:, 3 * C:4 * C],
                             rhs=ident_bf[:], start=True, stop=True)
            wvT_sb = mid.tile([C, C], BF16)
            nc.vector.tensor_copy(wvT_sb[:], wvT_ps[:])

            # ---- pass 1: Gram stats + PE transpose of x ------------------
            xT_store = singles.tile([C, NTOK], BF16)

            p1_ctx = ExitStack()
            psum_xt = p1_ctx.enter_context(
                tc.tile_pool(name="psum_xt", bufs=4, space="PSUM"))

            def xt_evac(base, xt_ps, engine):
                # PSUM reads run at ~1 elem/cycle/engine: alternate whole-
                # group copies between Vector and Act.
                if engine == 0:
                    nc.vector.tensor_copy(xT_store[:, base:base + GW],
                                          xt_ps[:])
                else:
                    nc.scalar.copy(xT_store[:, base:base + GW], xt_ps[:])

            for ch in range(NCH):
                if ch == NCH - 1:
                    # first half inline (gram+transpose), then close the S
                    # accumulation with 8 gram-only tiles so S lands early;
                    # the last 8 tiles' transposes weave into the middle.
                    for grp in range(2):
                        xt_ps = psum_xt.tile([C, GW], BF16)
                        for k in range(GRP):
                            n = grp * GRP + k
                            g = ch * CHUNK + n
                            src, nn = xtile(g)
                            nc.tensor.matmul(
                                s_ps[:], lhsT=src[:, nn, 0:C],
                                rhs=src[:, nn, 0:C + 1],
                                start=False, stop=False)
                            nc.tensor.transpose(
                                xt_ps[:, k * 128:(k + 1) * 128],
                                src[:, nn, 0:C], ident_bf[:])
                        xt_evac((ch * CHUNK + grp * GRP) * 128, xt_ps, 0)
                    for n in range(2 * GRP, CHUNK):
                        g = ch * CHUNK + n
                        src, nn = xtile(g)
                        nc.tensor.matmul(
                            s_ps[:], lhsT=src[:, nn, 0:C],
                            rhs=src[:, nn, 0:C + 1],
                            start=False, stop=(g == NT - 1))
                else:
                    for grp in range(NGRP):
                        # bf16 transpose output: halves the PSUM bytes the
                        # evacuation engines read (is_transpose keeps lhsT
                        # dtype end-to-end).
                        xt_ps = psum_xt.tile([C, GW], BF16)
                        for k in range(GRP):
                            n = grp * GRP + k
                            g = ch * CHUNK + n
                            src, nn = xtile(g)
                            lhsT = src[:, nn, 0:C]
                            rhs = src[:, nn, 0:C + 1]
                            nc.tensor.matmul(s_ps[:], lhsT=lhsT, rhs=rhs,
                                             start=(g == 0), stop=False)
                            nc.tensor.transpose(
                                xt_ps[:, k * 128:(k + 1) * 128],
                                lhsT, ident_bf[:])
                        # ALL pass-1 evacs on Vector (bf16 transposes made
                        # them cheap enough): Act issues no Copy ops during
                        # pass 1, so the Sqrt table loaded at startup stays
                        # valid and the middle's sqrts run data-gated.
                        xt_evac((ch * CHUNK + grp * GRP) * 128, xt_ps, 0)

            def t_batch(grp):
                # one deferred transpose group of the last chunk
                xt_ps = psum_xt.tile([C, GW], BF16)
                for k in range(GRP):
                    g = (NCH - 1) * CHUNK + grp * GRP + k
                    src, nn = xtile(g)
                    nc.tensor.transpose(
                        xt_ps[:, k * 128:(k + 1) * 128],
                        src[:, nn, 0:C], ident_bf[:])
                return ((NCH - 1) * CHUNK + grp * GRP) * 128, xt_ps

            # ---- middle: attention matrix -> Wf, bf ----------------------
            s_bf = mid.tile([C, C + 1], BF16)
            nc.vector.tensor_copy(s_bf[:], s_ps[:])

            # SW = S @ [Wq | Wk]  (S symmetric)
            sw_ps = psum_mid.tile([C, 2 * C], F32, tag="mps")
            nc.tensor.matmul(sw_ps[:], lhsT=s_bf[:, 0:C], rhs=wpack[:, 0:2 * C],
                             start=True, stop=True)
            tb0 = t_batch(2)
            # srow = s^T [Wq | Wk] (rank-1 terms of G)
            srow_ps = psum_mid.tile([1, 2 * C], F32, tag="mps")
            nc.tensor.matmul(srow_ps[:], lhsT=s_bf[:, C:C + 1],
                             rhs=wpack[:, 0:2 * C], start=True, stop=True)
            tb1 = t_batch(3)
            # k-side first everywhere: the rk chain (sqk -> sqrt -> recip ->
            # transpose -> broadcast) is the middle's critical path; the q
            # side is only needed at the stt much later.
            sw_sb = mid.tile([C, 2 * C], BF16)
            nc.vector.tensor_copy(sw_sb[:, C:2 * C], sw_ps[:, C:2 * C])
            prod_sb = mid.tile([C, 2 * C], BF16)
            nc.vector.tensor_mul(prod_sb[:, C:2 * C], wpack[:, 6 * C:7 * C],
                                 sw_sb[:, C:2 * C])
            nc.vector.tensor_copy(sw_sb[:, 0:C], sw_ps[:, 0:C])
            nc.vector.tensor_mul(prod_sb[:, 0:C], wpack[:, 5 * C:6 * C],
                                 sw_sb[:, 0:C])
            srowkn_bf = mid.tile([1, C], BF16)
            nc.vector.tensor_add(srowkn_bf[:], srow_ps[:, C:2 * C],
                                 nkb_row[:])
            srowq_bf = mid.tile([1, C], BF16)
            nc.vector.tensor_copy(srowq_bf[:], srow_ps[:, 0:C])

            # sq columns: colsum(W .* SW) + (W*2b)^T s + N*b^2, k then q
            # in separate PSUM tiles so the k sqrt doesn't wait on q.
            sqk_ps = psum_mid.tile([C, 1], F32, tag="mps")
            nc.tensor.matmul(sqk_ps[:], lhsT=prod_sb[:, C:2 * C],
                             rhs=ones_col_bf[:], start=True, stop=False,
                             skip_group_check=True)
            nc.tensor.matmul(sqk_ps[:], lhsT=wpack[:, 2 * C:3 * C],
                             rhs=s_bf[:, C:C + 1], start=False, stop=False,
                             skip_group_check=True)
            nc.tensor.matmul(sqk_ps[:], lhsT=rowpack[:, 3 * C:4 * C],
                             rhs=one_one_bf[:], start=False, stop=True,
                             skip_group_check=True)
            sqq_ps = psum_mid.tile([C, 1], F32, tag="mps")
            nc.tensor.matmul(sqq_ps[:], lhsT=prod_sb[:, 0:C],
                             rhs=ones_col_bf[:], start=True, stop=False,
                             skip_group_check=True)
            nc.tensor.matmul(sqq_ps[:], lhsT=wpack[:, 4 * C:5 * C],
                             rhs=s_bf[:, C:C + 1], start=False, stop=False,
                             skip_group_check=True)
            nc.tensor.matmul(sqq_ps[:], lhsT=rowpack[:, 2 * C:3 * C],
                             rhs=one_one_bf[:], start=False, stop=True,
                             skip_group_check=True)

            # G = mask + Wq^T S Wk + qb (x) (srow_k + N*kb) + (Wq^T s)(x) kb
            # mask terms go FIRST so the in-block +-2^100 cancel exactly
            # before the (much smaller) G terms accumulate.
            g_ps = psum_mid.tile([C, C], F32, tag="mps")
            nc.tensor.matmul(g_ps[:], lhsT=mrow_neg[:], rhs=mrow_pos[:],
                             start=True, stop=False, skip_group_check=True)
            for h in range(HEADS):
                hr = hrows[:, h * C:(h + 1) * C]
                nc.tensor.matmul(g_ps[:], lhsT=hr, rhs=hr,
                                 start=False, stop=False,
                                 skip_group_check=True)
            nc.tensor.matmul(g_ps[:], lhsT=wpack[:, 0:C],
                             rhs=sw_sb[:, C:2 * C], start=False, stop=False,
                             skip_group_check=True)
            nc.tensor.matmul(g_ps[:], lhsT=rowpack[:, 0:C], rhs=srowkn_bf[:],
                             start=False, stop=False, skip_group_check=True)
            nc.tensor.matmul(g_ps[:], lhsT=srowq_bf[:], rhs=rowpack[:, C:2 * C],
                             start=False, stop=True, skip_group_check=True)
            keepalive(n=3)

            # rq = exp(scale)/sqrt(sqq) = rsqrt(sqq*e2); rk = rsqrt(sqk).
            # EPS guard dropped (sq >> EPS always here).
            sqk_s = mid.tile([C, 1], F32)
            nc.scalar.sqrt(sqk_s[:], sqk_ps[:])
            sqq_s = mid.tile([C, 1], F32)
            nc.scalar.sqrt(sqq_s[:], sqq_ps[:])
            # preload the Exp table while the reciprocal/rk chain runs
            nc.scalar.activation(act_warm[:], act_warm[:],
                                 mybir.ActivationFunctionType.Exp)
            rk_col = mid.tile([C, 1], BF16)
            rq_col = mid.tile([C, 1], BF16)
            with nc.allow_low_precision(reason="rq/rk are softmax scales"):
                nc.vector.reciprocal(rk_col[:], sqk_s[:])
            # g -> SBUF early (off-critical, f32 to keep logits exact):
            # lets the stt read the rk broadcast straight from PSUM,
            # skipping a copy hop.
            g_sb = mid.tile([C, C], F32)
            nc.vector.tensor_copy(g_sb[:], g_ps[:])
            with nc.allow_low_precision(reason="rq/rk are softmax scales"):
                nc.vector.reciprocal(rq_col[:], sqq_s[:])

            # rk column -> row -> broadcast to all partitions
            rkr_ps = psum_mid.tile([1, C], F32, tag="mps")
            nc.tensor.matmul(rkr_ps[:], lhsT=rk_col[:], rhs=ident_bf[:],
                             start=True, stop=True)
            rk_row = mid.tile([1, C], BF16)
            nc.vector.tensor_copy(rk_row[:], rkr_ps[:])
            rkb_ps = psum_mid.tile([C, C], F32, tag="mps")
            nc.tensor.matmul(rkb_ps[:], lhsT=ones_row_bf[:], rhs=rk_row[:],
                             start=True, stop=True)
            # keepalives fill the PE idle under the softmax chain (HAM MID
            # re-gates if duty drops too low)
            keepalive(n=6)

            # masked softmax, no max-subtraction: bias = -exp(scale) bounds
            # the exponent at 0 (Cauchy-Schwarz), 1/rowsum folds into proj_w.
            # logits = (rk_bc * rq) * g, mask already inside g.
            logits = mid.tile([128, 128], F32)
            nc.vector.scalar_tensor_tensor(
                logits[:], rkb_ps[:], rq_col[:, 0:1], g_sb[:],
                op0=mybir.AluOpType.mult, op1=mybir.AluOpType.mult)
            # deferred evacuation fills the DVE gap under exp
            xt_evac(tb0[0], tb0[1], 0)
            attn_big = mid.tile([128, 128], BF16)
            sumx = mid.tile([128, 1], F32)
            nc.scalar.activation(attn_big[:], logits[:],
                                 mybir.ActivationFunctionType.Exp,
                                 bias=colpack[:, 1:2], accum_out=sumx[:])
            keepalive(lhs=attn_big[:, 0:1], n=2)
            rs = mid.tile([128, 1], F32)
            nc.vector.reciprocal(rs[:], sumx[:])
            pw_scaled = mid.tile([C, C], BF16)
            nc.vector.tensor_scalar(pw_scaled[:], pw_sb[:], rs[:, 0:1], None,
                                    op0=mybir.AluOpType.mult)

            # P = blockdiag(A)^T @ (pw/rowsum) ; Wf = Wv P ; bf = P^T vb + pb
            p_ps = psum_mid.tile([C, C], F32, tag="mps")
            nc.tensor.matmul(p_ps[:], lhsT=attn_big[:], rhs=pw_scaled[:],
                             start=True, stop=True)
            keepalive(lhs=attn_big[:, 1:2], n=2)
            p_sb = mid.tile([C, C], BF16)
            nc.scalar.copy(p_sb[:], p_ps[:])

            wf_ps = psum_mid.tile([C, C], F32, tag="mps")
            nc.tensor.matmul(wf_ps[:], lhsT=wvT_sb[:], rhs=p_sb[:],
                             start=True, stop=True)
            bf_ps = psum_mid.tile([C, 1], F32, tag="mps")
            nc.tensor.matmul(bf_ps[:], lhsT=p_sb[:], rhs=vb_col[:],
                             start=True, stop=True)
            wf_bf = mid.tile([C, C], BF16)
            nc.vector.tensor_copy(wf_bf[:], wf_ps[:])
            bf_col = mid.tile([C, 1], F32)
            nc.vector.tensor_add(bf_col[:], bf_ps[:], pb_col[:])
            # last deferred group evacuates on Act behind its exp/p_sb
            xt_evac(tb1[0], tb1[1], 1)

            # ---- pass 2: Y^T = Wf^T X^T + bf (per-partition bias) --------
            p1_ctx.close()
            mid_ctx.close()
            ctx.close()
            # output DMA blocks in pass-2 matmul pairs (1024 tokens each):
            # a small 256KB first block so the out-DMA starts ASAP, then
            # 512KB blocks; the final 2 matmuls get single-width copies on
            # both engines + 128KB DMAs to minimize the tail. Each pair's
            # PSUM evacuation is split across Vector and Act (512 cols
            # apiece) so both engines drain every pair.
            NBLK = NTOK // (4 * P2N)  # 8 blocks of 2048 tokens
            with (
                tc.tile_pool(name="yout", bufs=4, space="SBUF") as yout_pool,
                tc.tile_pool(name="psum_y", bufs=3, space="PSUM") as psum_y,
                tc.tile_pool(name="psum_ka", bufs=1, space="PSUM") as psum_ka,
            ):
                ka_ps = psum_ka.tile([128, 2 * 128], F32)
                for b in range(NBLK):
                    yout = yout_pool.tile([C, 4 * P2N], BF16)
                    for pp in range(2):
                        p = 2 * b + pp
                        y_ps = psum_y.tile([128, 2 * P2N], F32, tag="yps")
                        for h in range(2):
                            j = 2 * p + h
                            nc.tensor.matmul(
                                y_ps[:, h * P2N:(h + 1) * P2N], lhsT=wf_bf[:],
                                rhs=xT_store[:, j * P2N:(j + 1) * P2N],
                                start=True, stop=True, skip_group_check=True)
                        if p >= 5:
                            # pass 2 is DMA-bound (~60% PE duty): without
                            # filler the HAM MID window keeps re-gating the
                            # clock mid-pass. Two dummy 256-col matmuls per
                            # pair hold duty high; skipped for the first
                            # (cold, PE-bound) pairs where they would hurt.
                            for _ in range(2):
                                nc.tensor.matmul(
                                    ka_ps[:], lhsT=wf_bf[:],
                                    rhs=xT_store[:, 0:2 * 128],
                                    start=True, stop=True,
                                    skip_group_check=True)
                        o = 2 * pp * P2N
                        nc.vector.tensor_scalar(
                            yout[:, o:o + P2N], y_ps[:, 0:P2N],
                            bf_col[:, 0:1], None, op0=mybir.AluOpType.add)
                        nc.scalar.activation(
                            yout[:, o + P2N:o + 2 * P2N], y_ps[:, P2N:2 * P2N],
                            mybir.ActivationFunctionType.Identity,
                            bias=bf_col[:, 0:1])
                        # half-block (256KB) DMAs throughout: the write
                        # stream starts sooner after each pair and the tail
                        # drains sooner than with monolithic 512KB blocks.
                        nc.sync.dma_start(
                            out_d.ap()[b, :, o:o + 2 * P2N],
                            yout[:, o:o + 2 * P2N])

    nc.compile()
    return nc


def kernel(x, qkv_w, q_bias, v_bias, scale, proj_w, proj_b, num_heads=4):
    global _CACHED_NC, LAST_EXEC_TIME_NS
    _install_ntff_hook()
    if _CACHED_NC is None:
        _CACHED_NC = build()
    nc = _CACHED_NC

    BF = ml_dtypes.bfloat16
    x = np.asarray(x, dtype=np.float32)
    qkv_w = np.asarray(qkv_w, dtype=np.float32)
    q_bias = np.asarray(q_bias, dtype=np.float32)
    v_bias = np.asarray(v_bias, dtype=np.float32)
    scale = np.asarray(scale, dtype=np.float32).reshape(HEADS)
    proj_w = np.asarray(proj_w, dtype=np.float32)
    proj_b = np.asarray(proj_b, dtype=np.float32)

    # reference reshapes qkv to (..., heads, 3, hd): column (h, t, d) of qkv_w
    # is h*96 + t*32 + d, and bias384 = concat(q_bias, 0, v_bias) is applied
    # in that interleaved order. Permute host-side to [Wq | Wk | Wv] blocks
    # with matching effective biases (k picks up a nonzero bias).
    idx = np.concatenate([np.arange(h * 3 * HD, h * 3 * HD + HD)
                          for h in range(HEADS)])
    bias384 = np.concatenate([q_bias, np.zeros_like(q_bias), v_bias])
    wq = qkv_w[:, idx]
    wk = qkv_w[:, idx + HD]
    wv = qkv_w[:, idx + 2 * HD]
    qbe, kbe, vbe = bias384[idx], bias384[idx + HD], bias384[idx + 2 * HD]
    n_f = np.float32(NTOK)
    e2 = np.repeat(np.exp(-2.0 * scale), HD).astype(np.float32)

    wpack = np.concatenate(
        [wq, wk, wk * (2.0 * kbe)[None, :], wv,
         wq * (2.0 * qbe * e2)[None, :], wq * e2[None, :], wk], axis=1)
    rowpack = np.concatenate(
        [qbe, kbe, n_f * qbe * qbe * e2, n_f * kbe * kbe])[None, :]
    colpack = np.stack(
        [n_f * qbe * qbe * e2,
         -np.repeat(np.exp(scale), HD),
         n_f * kbe * kbe], axis=1).astype(np.float32)

    # Host-side token permutation: the kernel stores PE-transposed columns in
    # (chunk, tile, partition) order; permute input rows so that order is the
    # true token order and the output DMA is fully linear.
    xr = x.reshape(B, NDC, DCN, 128, C).transpose(0, 1, 3, 2, 4)
    xpad = np.zeros((B, NTOK, XCOL), dtype=BF)
    xpad[:, :, 0:C] = xr.reshape(B, NTOK, C).astype(BF)
    xpad[:, :, C] = BF(1.0)

    shared = {
        "wpack": np.ascontiguousarray(wpack.astype(BF)),
        "rowpack": np.ascontiguousarray(rowpack.astype(BF)),
        "nkb_row": np.ascontiguousarray((n_f * kbe)[None, :]),
        "pb_col": np.ascontiguousarray(proj_b[:, None]),
        "vb_col": np.ascontiguousarray(vbe[:, None].astype(BF)),
        "colpack": np.ascontiguousarray(colpack),
        "proj_w": np.ascontiguousarray(proj_w.astype(BF)),
    }
    in_maps = [
        {"x": np.ascontiguousarray(xpad[i]), **shared}
        for i in range(B)
    ]
    trace = bool(os.environ.get("BASS_TRACE"))
    res = run_bass_kernel_spmd(nc, in_maps, core_ids=list(range(B)),
                               trace=trace)
    LAST_EXEC_TIME_NS = res.exec_time_ns
    out = np.stack([
        res.results[i]["out"].astype(np.float32)
        .transpose(1, 0, 2).reshape(C, NTOK).T.reshape(H, W, C)
        for i in range(B)
    ])
    return out

